# revision 16
# baseline (speedup 1.0000x reference)
"""Trainium2 Bass kernel: BlockAttnRes forward.

Reference computation (per batch b, position t):
    k[n]   = s[n] / sqrt(mean(s[n]^2) + eps)        n in [0, 9)
    score  = k[n] . w                                (w = queries[layer_idx])
    alpha  = softmax(score over n)
    h[t]   = sum_n alpha[n] * s[n]                   (d = 512)

Distribution: batch dim B=8 -> one batch per NeuronCore (8 cores), no
cross-core communication.  Per core: T=4096 positions processed in 32
tiles of 128 (partition dim = position).

The kernel is HBM-bandwidth bound: 72 MB read + 8 MB written per core
(~8 us/tile measured DMA active).  Everything else is sized to stay
under that:

    DMA   : ONE SWDGE (gpsimd) dma per tile loading [128, 9, 512] with an
            inline f32->fp16 cast (HBM read traffic unchanged; SBUF fill
            halved, all engines see fp16).  fp16 rather than bf16: the
            softmax is decided by dot-products of ~N(0, 22) scores, and
            bf16's 8-bit mantissa perturbs scores by ~0.05 -- enough to
            reshuffle near-tied alphas (measured 7e-2 rel err).  fp16's
            11-bit mantissa lands ~1e-2.  Output h via HWDGE (sync).
    ACT   : ssq[n] = sum_d s^2 for n < SSQ_ACT_K (Square + accum_out)
            rsq = Exp(-0.5 * Ln(ssq/512 + eps)); e = Exp(score - max)
    DVE   : dot[n] = sum_d s*w for all n (STT + accum; ACT cannot
            multiply two tensors, GPSIMD's ISA rejects STT); remaining
            ssq n's; softmax smalls; final h = (1/sum_e) * h_psum
            (tensor_scalar, doubles as the PSUM->SBUF move).
    GPSIMD: builds diag(e_n) [128, 9*128] with ONE local_scatter (the
            ucode zeroes the dst and writes e at per-partition indices
            128*n + p -- a diagonal write no AP on DVE/ACT can express).
            Also hosts the SWDGE input-dma triggers, issued PREFETCH
            tiles ahead of use.
    PE    : h_psum += diag(e_n).T @ s_n, 9 accumulating fp16 matmuls
            (1-pass, FWL) -- ~3x cheaper than the fp32 pair lowering.

The softmax is computed unnormalized (diag of raw e = exp(score-max));
the single 1/sum_e scale at the end normalizes, saving the per-n alpha
multiply and the separate final STT.

All ACT functions (square, ln, exp) live in the single
`natural_log_exp_and_others` table set -> one ACT_TABLE_LOAD total
(pinned via PinnedBacc below; the stock chooser thrashes sets).
"""

import numpy as np

B, T, N, D = 8, 4096, 9, 512
P = 128
EPS = 1e-6
NCORES = 8

# how many of the 9 per-n ssq reductions run on ACT (rest on DVE)
SSQ_ACT_K = 8
PREFETCH = 6
NSPLIT = 5  # n's in the first of the two per-tile input DMAs

_CACHE = {}


def _build_bass(t_len=T):
    import concourse.bass as bass
    import concourse.tile as tile
    from concourse import bacc, mybir

    f32 = mybir.dt.float32
    fp16 = mybir.dt.float16
    i16 = mybir.dt.int16
    Alu = mybir.AluOpType
    Act = mybir.ActivationFunctionType
    Ax = mybir.AxisListType

    ntiles = t_len // P

    # Bacc (not raw Bass): its compile() pass splits multi-sem waits into
    # InstEventSemaphore -- TRN2 engine instructions hold at most ONE wait.
    PINNED_SET = "natural_log_exp_and_others"

    class PinnedBacc(bacc.Bacc):
        def insert_act_table_loads(self):
            import bass_rust as _bass_rust
            from concourse.hw_specs import get_activation_tables

            all_tables = get_activation_tables(self.m.arch)
            used = {
                i.func
                for b in self.main_func.blocks
                for i in b.instructions
                if isinstance(i, mybir.InstActivation)
            }
            if used and PINNED_SET in all_tables and used <= all_tables[PINNED_SET]:
                tables = [
                    (name, funcs if name == PINNED_SET else set())
                    for name, funcs in all_tables.items()
                ]
            else:
                tables = list(all_tables.items())
            _bass_rust.insert_act_table_loads(self, tables)

    nc = PinnedBacc("TRN2", target_bir_lowering=False, debug=False)
    src = nc.dram_tensor("src", [t_len, N, D], f32, kind="ExternalInput").ap()
    wq = nc.dram_tensor("wq", [P, D], f32, kind="ExternalInput").ap()
    out = nc.dram_tensor("out", [t_len, D], f32, kind="ExternalOutput").ap()

    src_t = src.rearrange("(c p) n d -> c p n d", p=P)
    out_t = out.rearrange("(c p) d -> c p d", p=P)

    with tile.TileContext(nc) as tc:
        with (
            tc.tile_pool(name="const", bufs=1) as const_pool,
            tc.tile_pool(name="srcp", bufs=PREFETCH + 4) as src_pool,
            tc.tile_pool(name="scratch", bufs=2) as scr_pool,
            tc.tile_pool(name="small", bufs=8) as small_pool,
            tc.tile_pool(name="diag", bufs=4) as diag_pool,
            tc.tile_pool(name="hout", bufs=6) as out_pool,
            tc.tile_pool(name="psum", bufs=6, space="PSUM") as psum_pool,
        ):
            # w, cast to fp16 during the load
            w_sb = const_pool.tile([P, D], fp16)
            nc.gpsimd.dma_start(out=w_sb, in_=wq)
            eps_sb = const_pool.tile([P, 1], f32)
            nc.vector.memset(eps_sb, EPS)

            # scatter indices for diag(e): idx[p, n] = 128*n + p for n < 9,
            # idx[p, 9] = -1 (padding; negative = ignored by local_scatter)
            idx_sb = const_pool.tile([P, N + 1], i16)
            nc.gpsimd.iota(
                idx_sb[:, :N], pattern=[[P, N]], base=0, channel_multiplier=1
            )
            nc.gpsimd.memset(idx_sb[:, N : N + 1], -1)

            # Software-pipelined schedule, 3 stages deep.  Every engine's
            # queue is strict FIFO, so an op whose producer (on another
            # engine) runs in the same iteration stalls the whole queue
            # behind it.  Emitting each dependent stage one iteration later
            # makes every cross-engine dependency >= 1 tile old by the time
            # its engine reaches it -- zero steady-state stalls.
            #
            #   iter c    stage A (tile c):   loads, ssq, dot, rsq
            #             stage B (tile c-1): score, nmx
            #             stage C (tile c-2): e, sum_e, 1/sum_e, scatter, MMs
            #             stage D (tile c-3): hs, store
            st = {}

            def issue_load(c, fine=False):
                # two chunk DMAs into one tile: consumers of low n's start
                # ~3.5us earlier, and the SWDGE generation pipelines better.
                # The first tiles load per-n (fine) so the compute ramp
                # starts at ~1us instead of waiting for a whole chunk.
                sk = src_pool.tile([P, N, D], fp16, tag="s")
                if fine:
                    for n in range(N):
                        nc.gpsimd.dma_start(
                            out=sk[:, n, :], in_=src_t[c, :, n, :]
                        )
                else:
                    nc.gpsimd.dma_start(
                        out=sk[:, :NSPLIT, :], in_=src_t[c, :, :NSPLIT, :]
                    )
                    nc.gpsimd.dma_start(
                        out=sk[:, NSPLIT:, :], in_=src_t[c, :, NSPLIT:, :]
                    )
                st[c] = {"s": sk}

            def stage_a(c):
                t = st[c]
                s = t["s"]
                ssq = small_pool.tile([P, N], f32, tag="ssq")
                t["ssq"] = ssq
                dot = small_pool.tile([P, N], f32, tag="dot")
                t["dot"] = dot

                # DVE: dots for the first-chunk n's before anything that
                # needs the second chunk, so the ramp-in isn't gated on the
                # whole tile; then the n=8 ssq (so ssq completes before ACT,
                # 8 squares later, reaches the Ln), then the remaining dots
                pr_v = scr_pool.tile([P, D], fp16, tag="pr_v")
                sq_v = scr_pool.tile([P, D], fp16, tag="sq_v")

                def dot_op(n):
                    nc.vector.scalar_tensor_tensor(
                        out=pr_v,
                        in0=s[:, n, :],
                        scalar=0.0,
                        in1=w_sb,
                        op0=Alu.bypass,
                        op1=Alu.mult,
                        accum_out=dot[:, n : n + 1],
                    )

                for n in range(NSPLIT):
                    dot_op(n)
                for n in range(SSQ_ACT_K, N):
                    nc.vector.scalar_tensor_tensor(
                        out=sq_v,
                        in0=s[:, n, :],
                        scalar=0.0,
                        in1=s[:, n, :],
                        op0=Alu.bypass,
                        op1=Alu.mult,
                        accum_out=ssq[:, n : n + 1],
                    )
                for n in range(NSPLIT, N):
                    dot_op(n)

                sq_a = scr_pool.tile([P, D], fp16, tag="sq_a")
                for n in range(SSQ_ACT_K):
                    nc.scalar.activation(
                        out=sq_a,
                        in_=s[:, n, :],
                        func=Act.Square,
                        accum_out=ssq[:, n : n + 1],
                    )

                # rsq = (ssq/D + eps)^(-1/2) via Exp(-0.5 * Ln(x))
                rsq = small_pool.tile([P, N], f32, tag="rsq")
                t["rsq"] = rsq
                nc.scalar.activation(
                    out=rsq, in_=ssq, func=Act.Ln, scale=1.0 / D, bias=eps_sb
                )
                nc.scalar.activation(out=rsq, in_=rsq, func=Act.Exp, scale=-0.5)

            def stage_b(c):
                t = st[c]
                score = small_pool.tile([P, N], f32, tag="score")
                t["score"] = score
                nc.vector.tensor_mul(score, t["dot"], t["rsq"])
                nmx = small_pool.tile([P, 1], f32, tag="nmx")
                t["nmx"] = nmx
                nc.vector.tensor_reduce(
                    out=nmx, in_=score, axis=Ax.X, op=Alu.max, negate=True
                )

            def stage_c_act(c):
                # e = exp(score - max), kept UNNORMALIZED (fp16 for the
                # scatter); normalization happens once at the end via rs
                t = st[c]
                e = small_pool.tile([P, N + 1], fp16, tag="e")
                t["e"] = e
                nc.scalar.activation(
                    out=e[:, :N], in_=t["score"], func=Act.Exp, bias=t["nmx"]
                )

            def stage_c_rest(c):
                t = st[c]
                e = t["e"]
                sume = small_pool.tile([P, 1], f32, tag="sume")
                nc.vector.tensor_reduce(
                    out=sume, in_=e[:, :N], axis=Ax.X, op=Alu.add
                )
                rs = small_pool.tile([P, 1], f32, tag="rs")
                t["rs"] = rs
                nc.vector.reciprocal(out=rs, in_=sume)

                # diag(e_n) built by GPSIMD: zeroes dg and writes e[p, n] at
                # free-offset 128*n + p (per-partition indices)
                dg = diag_pool.tile([P, N * P], fp16, tag="dg")
                nc.gpsimd.local_scatter(
                    out_ap=dg,
                    data_ap=e,
                    idxs_ap=idx_sb,
                    channels=P,
                    num_elems=N * P,
                    num_idxs=N + 1,
                )

                # h_psum += diag(e_n).T @ s_n
                s = t["s"]
                hp = psum_pool.tile([P, D], f32, tag="hp")
                t["hp"] = hp
                for n in range(N):
                    nc.tensor.matmul(
                        hp,
                        dg[:, n * P : (n + 1) * P],
                        s[:, n, :],
                        start=(n == 0),
                        stop=(n == N - 1),
                    )

            def stage_d(c):
                # h = (1/sum_e) * h_psum  (tensor_scalar; doubles as the
                # PSUM -> SBUF move and the softmax normalization)
                t = st.pop(c)
                hs = out_pool.tile([P, D], f32, tag="hs")
                nc.vector.tensor_scalar_mul(hs, t["hp"], t["rs"])
                nc.sync.dma_start(out=out_t[c], in_=hs)

            for c in range(min(PREFETCH, ntiles)):
                issue_load(c, fine=(c < 2))

            for c in range(ntiles + 3):
                if c < ntiles and c + PREFETCH < ntiles:
                    issue_load(c + PREFETCH)
                if c >= 2 and c - 2 < ntiles:
                    stage_c_act(c - 2)
                if c < ntiles:
                    stage_a(c)
                if c >= 1 and c - 1 < ntiles:
                    stage_b(c - 1)
                if c >= 2 and c - 2 < ntiles:
                    stage_c_rest(c - 2)
                if c >= 3:
                    stage_d(c - 3)

    nc.compile()
    return nc


def _get_nc(t_len=T):
    key = (t_len,)
    if key not in _CACHE:
        _CACHE[key] = _build_bass(t_len)
    return _CACHE[key]


def _make_in_maps(sources, queries, layer_idx):
    sources = np.ascontiguousarray(np.asarray(sources, dtype=np.float32))
    queries = np.asarray(queries, dtype=np.float32)
    w = queries[int(layer_idx)]
    w_rep = np.ascontiguousarray(np.broadcast_to(w[None, :], (P, D)).astype(np.float32))
    return [
        {"src": np.ascontiguousarray(sources[b]), "wq": w_rep}
        for b in range(sources.shape[0])
    ]


def kernel(sources, queries, layer_idx):
    from concourse.bass_utils import run_bass_kernel_spmd

    nc = _get_nc()
    in_maps = _make_in_maps(sources, queries, layer_idx)
    res = run_bass_kernel_spmd(nc, in_maps, core_ids=list(range(NCORES)))
    return np.stack([res.results[b]["out"] for b in range(NCORES)], axis=0)


# revision 18
# speedup vs baseline: 1.0697x; 1.0697x over previous
"""Trainium2 Bass kernel: BlockAttnRes forward.

Reference computation (per batch b, position t):
    k[n]   = s[n] / sqrt(mean(s[n]^2) + eps)        n in [0, 9)
    score  = k[n] . w                                (w = queries[layer_idx])
    alpha  = softmax(score over n)
    h[t]   = sum_n alpha[n] * s[n]                   (d = 512)

Distribution: batch dim B=8 -> one batch per NeuronCore (8 cores), no
cross-core communication.  Per core: T=4096 positions processed in 32
tiles of 128 (partition dim = position).

The kernel is HBM-bandwidth bound: 72 MB read + 8 MB written per core
(~8 us/tile measured DMA active).  Everything else is sized to stay
under that:

    DMA   : ONE SWDGE (gpsimd) dma per tile loading [128, 9, 512] with an
            inline f32->fp16 cast (HBM read traffic unchanged; SBUF fill
            halved, all engines see fp16).  fp16 rather than bf16: the
            softmax is decided by dot-products of ~N(0, 22) scores, and
            bf16's 8-bit mantissa perturbs scores by ~0.05 -- enough to
            reshuffle near-tied alphas (measured 7e-2 rel err).  fp16's
            11-bit mantissa lands ~1e-2.  Output h via HWDGE (sync).
    ACT   : ssq[n] = sum_d s^2 for n < SSQ_ACT_K (Square + accum_out)
            rsq = Exp(-0.5 * Ln(ssq/512 + eps)); e = Exp(score - max)
    DVE   : dot[n] = sum_d s*w for all n (STT + accum; ACT cannot
            multiply two tensors, GPSIMD's ISA rejects STT); remaining
            ssq n's; softmax smalls; final h = (1/sum_e) * h_psum
            (tensor_scalar, doubles as the PSUM->SBUF move).
    GPSIMD: builds diag(e_n) [128, 9*128] with ONE local_scatter (the
            ucode zeroes the dst and writes e at per-partition indices
            128*n + p -- a diagonal write no AP on DVE/ACT can express).
            Also hosts the SWDGE input-dma triggers, issued PREFETCH
            tiles ahead of use.
    PE    : h_psum += diag(e_n).T @ s_n, 9 accumulating fp16 matmuls
            (1-pass, FWL) -- ~3x cheaper than the fp32 pair lowering.

The softmax is computed unnormalized (diag of raw e = exp(score-max));
the single 1/sum_e scale at the end normalizes, saving the per-n alpha
multiply and the separate final STT.

All ACT functions (square, ln, exp) live in the single
`natural_log_exp_and_others` table set -> one ACT_TABLE_LOAD total
(pinned via PinnedBacc below; the stock chooser thrashes sets).
"""

import numpy as np

B, T, N, D = 8, 4096, 9, 512
P = 128
EPS = 1e-6
NCORES = 8

# how many of the 9 per-n ssq reductions run on ACT (rest on DVE)
SSQ_ACT_K = 8
PREFETCH = 6
NSPLIT = 5  # n's in the first of the two per-tile input DMAs

_CACHE = {}


def _build_bass(t_len=T):
    import concourse.bass as bass
    import concourse.tile as tile
    from concourse import bacc, mybir

    f32 = mybir.dt.float32
    fp16 = mybir.dt.float16
    i16 = mybir.dt.int16
    Alu = mybir.AluOpType
    Act = mybir.ActivationFunctionType
    Ax = mybir.AxisListType

    ntiles = t_len // P

    # Bacc (not raw Bass): its compile() pass splits multi-sem waits into
    # InstEventSemaphore -- TRN2 engine instructions hold at most ONE wait.
    PINNED_SET = "natural_log_exp_and_others"

    class PinnedBacc(bacc.Bacc):
        def insert_act_table_loads(self):
            import bass_rust as _bass_rust
            from concourse.hw_specs import get_activation_tables

            all_tables = get_activation_tables(self.m.arch)
            used = {
                i.func
                for b in self.main_func.blocks
                for i in b.instructions
                if isinstance(i, mybir.InstActivation)
            }
            if used and PINNED_SET in all_tables and used <= all_tables[PINNED_SET]:
                tables = [
                    (name, funcs if name == PINNED_SET else set())
                    for name, funcs in all_tables.items()
                ]
            else:
                tables = list(all_tables.items())
            _bass_rust.insert_act_table_loads(self, tables)

    nc = PinnedBacc("TRN2", target_bir_lowering=False, debug=False)
    src = nc.dram_tensor("src", [t_len, N, D], f32, kind="ExternalInput").ap()
    wq = nc.dram_tensor("wq", [P, D], f32, kind="ExternalInput").ap()
    out = nc.dram_tensor("out", [t_len, D], f32, kind="ExternalOutput").ap()

    src_t = src.rearrange("(c p) n d -> c p n d", p=P)
    out_t = out.rearrange("(c p) d -> c p d", p=P)

    with tile.TileContext(nc) as tc:
        with (
            tc.tile_pool(name="const", bufs=1) as const_pool,
            tc.tile_pool(name="srcp", bufs=PREFETCH + 4) as src_pool,
            tc.tile_pool(name="scratch", bufs=2) as scr_pool,
            tc.tile_pool(name="small", bufs=8) as small_pool,
            tc.tile_pool(name="diag", bufs=4) as diag_pool,
            tc.tile_pool(name="hout", bufs=6) as out_pool,
            tc.tile_pool(name="psum", bufs=6, space="PSUM") as psum_pool,
        ):
            # w, cast to fp16 during the load
            w_sb = const_pool.tile([P, D], fp16)
            nc.gpsimd.dma_start(out=w_sb, in_=wq)
            eps_sb = const_pool.tile([P, 1], f32)
            nc.vector.memset(eps_sb, EPS)

            # scatter indices for diag(e): idx[p, n] = 128*n + p for n < 9,
            # idx[p, 9] = -1 (padding; negative = ignored by local_scatter)
            idx_sb = const_pool.tile([P, N + 1], i16)
            nc.gpsimd.iota(
                idx_sb[:, :N], pattern=[[P, N]], base=0, channel_multiplier=1
            )
            nc.gpsimd.memset(idx_sb[:, N : N + 1], -1)

            # Software-pipelined schedule, 3 stages deep.  Every engine's
            # queue is strict FIFO, so an op whose producer (on another
            # engine) runs in the same iteration stalls the whole queue
            # behind it.  Emitting each dependent stage one iteration later
            # makes every cross-engine dependency >= 1 tile old by the time
            # its engine reaches it -- zero steady-state stalls.
            #
            #   iter c    stage A (tile c):   loads, ssq, dot, rsq
            #             stage B (tile c-1): score, nmx
            #             stage C (tile c-2): e, sum_e, 1/sum_e, scatter, MMs
            #             stage D (tile c-3): hs, store
            st = {}

            def issue_load(c):
                # two chunk DMAs into one tile: consumers of low n's start
                # ~3.5us earlier, and the SWDGE generation pipelines better
                sk = src_pool.tile([P, N, D], fp16, tag="s")
                nc.gpsimd.dma_start(
                    out=sk[:, :NSPLIT, :], in_=src_t[c, :, :NSPLIT, :]
                )
                nc.gpsimd.dma_start(
                    out=sk[:, NSPLIT:, :], in_=src_t[c, :, NSPLIT:, :]
                )
                st[c] = {"s": sk}

            def stage_a(c):
                t = st[c]
                s = t["s"]
                ssq = small_pool.tile([P, N], f32, tag="ssq")
                t["ssq"] = ssq
                dot = small_pool.tile([P, N], f32, tag="dot")
                t["dot"] = dot

                # DVE: dots for the first-chunk n's before anything that
                # needs the second chunk, so the ramp-in isn't gated on the
                # whole tile; then the n=8 ssq (so ssq completes before ACT,
                # 8 squares later, reaches the Ln), then the remaining dots
                pr_v = scr_pool.tile([P, D], fp16, tag="pr_v")
                sq_v = scr_pool.tile([P, D], fp16, tag="sq_v")

                def dot_op(n):
                    nc.vector.scalar_tensor_tensor(
                        out=pr_v,
                        in0=s[:, n, :],
                        scalar=0.0,
                        in1=w_sb,
                        op0=Alu.bypass,
                        op1=Alu.mult,
                        accum_out=dot[:, n : n + 1],
                    )

                for n in range(NSPLIT):
                    dot_op(n)
                for n in range(SSQ_ACT_K, N):
                    nc.vector.scalar_tensor_tensor(
                        out=sq_v,
                        in0=s[:, n, :],
                        scalar=0.0,
                        in1=s[:, n, :],
                        op0=Alu.bypass,
                        op1=Alu.mult,
                        accum_out=ssq[:, n : n + 1],
                    )
                for n in range(NSPLIT, N):
                    dot_op(n)

                sq_a = scr_pool.tile([P, D], fp16, tag="sq_a")
                for n in range(SSQ_ACT_K):
                    nc.scalar.activation(
                        out=sq_a,
                        in_=s[:, n, :],
                        func=Act.Square,
                        accum_out=ssq[:, n : n + 1],
                    )

                # rsq = (ssq/D + eps)^(-1/2) via Exp(-0.5 * Ln(x))
                rsq = small_pool.tile([P, N], f32, tag="rsq")
                t["rsq"] = rsq
                nc.scalar.activation(
                    out=rsq, in_=ssq, func=Act.Ln, scale=1.0 / D, bias=eps_sb
                )
                nc.scalar.activation(out=rsq, in_=rsq, func=Act.Exp, scale=-0.5)

            def stage_b(c):
                t = st[c]
                score = small_pool.tile([P, N], f32, tag="score")
                t["score"] = score
                nc.vector.tensor_mul(score, t["dot"], t["rsq"])
                nmx = small_pool.tile([P, 1], f32, tag="nmx")
                t["nmx"] = nmx
                nc.vector.tensor_reduce(
                    out=nmx, in_=score, axis=Ax.X, op=Alu.max, negate=True
                )

            def stage_c_act(c):
                # e = exp(score - max), kept UNNORMALIZED (fp16 for the
                # scatter); normalization happens once at the end via rs
                t = st[c]
                e = small_pool.tile([P, N + 1], fp16, tag="e")
                t["e"] = e
                nc.scalar.activation(
                    out=e[:, :N], in_=t["score"], func=Act.Exp, bias=t["nmx"]
                )

            def stage_c_rest(c):
                t = st[c]
                e = t["e"]
                sume = small_pool.tile([P, 1], f32, tag="sume")
                nc.vector.tensor_reduce(
                    out=sume, in_=e[:, :N], axis=Ax.X, op=Alu.add
                )
                rs = small_pool.tile([P, 1], f32, tag="rs")
                t["rs"] = rs
                nc.vector.reciprocal(out=rs, in_=sume)

                # diag(e_n) built by GPSIMD: zeroes dg and writes e[p, n] at
                # free-offset 128*n + p (per-partition indices)
                dg = diag_pool.tile([P, N * P], fp16, tag="dg")
                nc.gpsimd.local_scatter(
                    out_ap=dg,
                    data_ap=e,
                    idxs_ap=idx_sb,
                    channels=P,
                    num_elems=N * P,
                    num_idxs=N + 1,
                )

                # h_psum += diag(e_n).T @ s_n
                s = t["s"]
                hp = psum_pool.tile([P, D], f32, tag="hp")
                t["hp"] = hp
                for n in range(N):
                    nc.tensor.matmul(
                        hp,
                        dg[:, n * P : (n + 1) * P],
                        s[:, n, :],
                        start=(n == 0),
                        stop=(n == N - 1),
                    )

            def stage_d(c):
                # h = (1/sum_e) * h_psum  (tensor_scalar; doubles as the
                # PSUM -> SBUF move and the softmax normalization)
                t = st.pop(c)
                hs = out_pool.tile([P, D], f32, tag="hs")
                nc.vector.tensor_scalar_mul(hs, t["hp"], t["rs"])
                nc.sync.dma_start(out=out_t[c], in_=hs)

            for c in range(min(PREFETCH, ntiles)):
                issue_load(c)

            for c in range(ntiles + 3):
                if c < ntiles and c + PREFETCH < ntiles:
                    issue_load(c + PREFETCH)
                if c >= 2 and c - 2 < ntiles:
                    stage_c_act(c - 2)
                if c < ntiles:
                    stage_a(c)
                if c >= 1 and c - 1 < ntiles:
                    stage_b(c - 1)
                if c >= 2 and c - 2 < ntiles:
                    stage_c_rest(c - 2)
                if c >= 3:
                    stage_d(c - 3)

    nc.compile()
    return nc


def _get_nc(t_len=T):
    key = (t_len,)
    if key not in _CACHE:
        _CACHE[key] = _build_bass(t_len)
    return _CACHE[key]


def _make_in_maps(sources, queries, layer_idx):
    sources = np.ascontiguousarray(np.asarray(sources, dtype=np.float32))
    queries = np.asarray(queries, dtype=np.float32)
    w = queries[int(layer_idx)]
    w_rep = np.ascontiguousarray(np.broadcast_to(w[None, :], (P, D)).astype(np.float32))
    return [
        {"src": np.ascontiguousarray(sources[b]), "wq": w_rep}
        for b in range(sources.shape[0])
    ]


def kernel(sources, queries, layer_idx):
    from concourse.bass_utils import run_bass_kernel_spmd

    nc = _get_nc()
    in_maps = _make_in_maps(sources, queries, layer_idx)
    res = run_bass_kernel_spmd(nc, in_maps, core_ids=list(range(NCORES)))
    return np.stack([res.results[b]["out"] for b in range(NCORES)], axis=0)
